# revision 20
# baseline (speedup 1.0000x reference)
"""Trainium2 Bass kernel for nn_CopyModel (gated linear-recurrence LM block).

Model: embed -> rmsnorm -> in_proj(1024->4*4096) -> sigmoid gates ->
linear scan h_t = a_t*h_{t-1} + b_t*x_t -> out gate c_t*h_t ->
out_proj(4096->1024) + residual -> head(1024->62).

Key insights:
 1. The vocab has only 62 entries, so every per-token quantity (embed,
    rmsnorm, in_proj, gate sigmoids) is a table lookup. The host precomputes
    per-vocab tables; the device gathers rows via one-hot matmuls.
 2. The output gate folds INTO the scan in log domain: with
    z_t := c_t*h_t,  z_t = exp(la[tok_t] + lc[tok_t] - lc[tok_{t-1}]) * z_{t-1}
                           + (c*bx)[tok_t]
    so the device never multiplies by c at all. The gate exponent is ONE
    matmul per 128-channel tile: a 124-partition stationary stacks the
    log(a) and log(c) tables, and the moving "two-hot" holds +1 at tok_t
    (both sections) and -1 at tok_{t-1} (log(c) section). The log(c)
    quantization telescopes exactly (same fp16 entry +/-), and log(a)'s
    fp16 error vanishes as a->1, so precision is safe. exp() runs on the
    otherwise-idle Act engine.
 3. Everything downstream of z is linear: out_proj and head fuse into
    out_wh = out_w @ head_w [4096, 62]; residual + biases commute with the
    head into a host epilogue.

Sharding: STATE (4096) split 8 ways (512 channels/core), both batches on
every core; the host sums the 8 partial logit contributions.

Per core, per 512-token chunk (8 chunks): PE: 4 gate-gathers + 4 out
matmuls (all fp16, 1 cyc/row); Act: 4 exp()s + 1 logits copy; DVE: scans
in [128, 1024] blocks (the pacing engine, ~2 ALU-cycles/element); Pool:
idle; DMA: c*bx stream + logits out.
"""

import sys

for _p in ("/opt/trn_rl_repo",):
    if _p not in sys.path:
        sys.path.insert(0, _p)

import numpy as np

import concourse.bass as bass
import concourse.bacc as bacc
import concourse.tile as tile
from concourse import mybir
from concourse.bass_utils import run_bass_kernel_spmd

F32 = mybir.dt.float32
F16 = mybir.dt.float16
AF = mybir.ActivationFunctionType
OP = mybir.AluOpType

V = 62          # vocab
VP = 128        # vocab padded to full partition count
H = 1024        # hidden
S = 4096        # state
B, L = 2, 2048
BL = B * L      # 4096 tokens
NCORES = 8
SS = S // NCORES        # 512 state channels per core
NST = SS // 128         # 4 state tiles per core
TC = 512                # tokens per chunk
NCHUNK = BL // TC       # 8 chunks
NBLK = NCHUNK // 2      # 4 scan blocks of 1024 tokens (2 per batch)
EPS = 1e-6


def _build_nc():
    nc = bacc.Bacc("TRN2", target_bir_lowering=False, debug=False)

    ohp_d = nc.dram_tensor("ohp", [VP, BL], F16, kind="ExternalInput")
    tab_d = nc.dram_tensor("tab", [VP, SS], F16, kind="ExternalInput")
    cbx_d = nc.dram_tensor("cbx", [128, NST * BL], F16, kind="ExternalInput")
    outwh_d = nc.dram_tensor("outwh", [128, NST * V], F16, kind="ExternalInput")
    logits = nc.dram_tensor("logits", [128, BL // 2], F16, kind="ExternalOutput")

    with tile.TileContext(nc) as tc:
        with (
            tc.tile_pool(name="consts", bufs=1) as consts,
            tc.tile_pool(name="p_a", bufs=2) as p_a,
            tc.tile_pool(name="p_z", bufs=2) as p_z,
            tc.tile_pool(name="p_lg", bufs=2) as p_lg,
            tc.tile_pool(name="psG", bufs=3, space="PSUM") as psG,
            tc.tile_pool(name="psL", bufs=2, space="PSUM") as psL,
        ):
            # ---- loads, critical-path first ----
            tab = consts.tile([VP, SS], F16)
            ohp = consts.tile([VP, BL], F16)
            cbx = consts.tile([128, NST * BL], F16)
            outwh = consts.tile([128, NST * V], F16)
            nc.sync.dma_start(out=tab[:], in_=tab_d[:])
            nc.sync.dma_start(out=ohp[:, 0:2 * TC], in_=ohp_d[:, 0:2 * TC])
            for st in range(NST):
                nc.sync.dma_start(
                    out=cbx[:, st * BL:st * BL + 2 * TC],
                    in_=cbx_d[:, st * BL:st * BL + 2 * TC],
                )
            nc.sync.dma_start(out=outwh[:], in_=outwh_d[:])
            o = 2 * TC
            nc.sync.dma_start(out=ohp[:, o:BL], in_=ohp_d[:, o:BL])
            for st in range(NST):
                ob = st * BL + o
                nc.sync.dma_start(out=cbx[:, ob:ob + BL - o], in_=cbx_d[:, ob:ob + BL - o])

            # ---- PE warmup: burn the p-state ramp during the DMA preamble ----
            gw = consts.tile([128, TC], F16)
            nc.vector.memset(gw[:], 0.0)
            for i in range(0):
                wps = psG.tile([128, TC], F32, tag="g")
                nc.tensor.matmul(
                    wps[:, 0:TC // 2], gw[:, 0:128], gw[:, 0:TC // 2],
                    start=True, stop=True,
                )

            def emit_gather_exp(b, ap_tiles, split=False):
                for st in range(NST):
                    pg = psG.tile([128, 2 * TC], F32, tag="g", name=f"pg{st}")
                    for half in range(2):
                        t0 = (2 * b + half) * TC
                        hs = slice(half * TC, (half + 1) * TC)
                        nc.tensor.matmul(
                            pg[:, hs],
                            tab[:, st * 128:(st + 1) * 128], ohp[:, t0:t0 + TC],
                            start=True, stop=True,
                        )
                        if split:
                            nc.scalar.activation(ap_tiles[st][:, hs], pg[:, hs], AF.Exp)
                    if not split:
                        nc.scalar.activation(ap_tiles[st][:], pg[:], AF.Exp)

            def new_ap():
                return [p_a.tile([128, 2 * TC], F32, tag=f"ap{st}", name=f"ap{st}")
                        for st in range(NST)]

            def emit_outs(b, zt):
                # both chunks of the block into one psum bank: even chunk at
                # partitions 0..61, odd chunk at 64..125 (PE tile_position)
                pl = psL.tile([128, TC], F32, tag="l")
                for half in range(2):
                    pb = 64 * half
                    for st in range(NST):
                        nc.tensor.matmul(
                            pl[pb:pb + V, :], outwh[:, st * V:(st + 1) * V],
                            zt[st][:, half * TC:(half + 1) * TC],
                            start=(st == 0), stop=(st == NST - 1),
                        )
                lg = p_lg.tile([128, TC], F16, tag="lg")
                nc.gpsimd.memset(lg[:], 0.0)
                nc.scalar.activation(lg[0:V, :], pl[0:V, :], AF.Copy)
                nc.scalar.activation(lg[64:64 + V, :], pl[64:64 + V, :], AF.Copy)
                nc.sync.dma_start(out=logits[:, b * TC:(b + 1) * TC], in_=lg[:])

            ap_cur = new_ap()
            emit_gather_exp(0, ap_cur, split=True)
            prev_z = [None] * NST
            for b in range(NBLK):
                # scans for block b: z = gate*z_prev + cbx along 1024 tokens
                reset = (b % (NBLK // B)) == 0
                last = b == NBLK - 1
                zt = [p_z.tile([128, 2 * TC], F16, tag=f"z{st}", name=f"z{st}")
                      for st in range(NST)]
                halves = 2 if (b == 0 or last) else 1
                for half in range(halves):
                    hs = (slice(half * TC, (half + 1) * TC) if halves == 2
                          else slice(0, 2 * TC))
                    for st in range(NST):
                        if half == 0:
                            init = 0.0 if reset else prev_z[st][:, 2 * TC - 1:2 * TC]
                        else:
                            init = zt[st][:, TC - 1:TC]
                        nc.vector.tensor_tensor_scan(
                            zt[st][:, hs], ap_cur[st][:, hs],
                            cbx[:, st * BL + b * 2 * TC + hs.start:
                                 st * BL + b * 2 * TC + hs.stop],
                            init, op0=OP.mult, op1=OP.add,
                        )
                    if last and half == 0:
                        # tail shortening: chunk-6 outs run while the final
                        # scans execute
                        pl_t = psL.tile([128, TC], F32, tag="l")
                        for st in range(NST):
                            nc.tensor.matmul(
                                pl_t[0:V, :], outwh[:, st * V:(st + 1) * V],
                                zt[st][:, 0:TC],
                                start=(st == 0), stop=(st == NST - 1),
                            )
                        lg_t = p_lg.tile([128, TC], F16, tag="lg")
                        nc.gpsimd.memset(lg_t[:], 0.0)
                        nc.scalar.activation(lg_t[0:V, :], pl_t[0:V, :], AF.Copy)
                prev_z = zt
                # PE/Act run a block ahead while DVE scans
                ap_next = None
                if b + 1 < NBLK:
                    ap_next = new_ap()
                    emit_gather_exp(b + 1, ap_next)
                if last:
                    for st in range(NST):
                        nc.tensor.matmul(
                            pl_t[64:64 + V, :], outwh[:, st * V:(st + 1) * V],
                            zt[st][:, TC:2 * TC],
                            start=(st == 0), stop=(st == NST - 1),
                        )
                    nc.scalar.activation(lg_t[64:64 + V, :], pl_t[64:64 + V, :], AF.Copy)
                    nc.sync.dma_start(out=logits[:, b * TC:(b + 1) * TC], in_=lg_t[:])
                else:
                    emit_outs(b, zt)
                ap_cur = ap_next

    nc.compile()
    return nc


_NC = None


def _get_nc():
    global _NC
    if _NC is None:
        _NC = _build_nc()
    return _NC


def _prep(tokens, embed_w, norm_w, in_w, in_b, out_w, out_b, head_w, head_b):
    tokens = np.asarray(tokens).reshape(-1)
    embed_w = np.asarray(embed_w, dtype=np.float32)
    norm_w = np.asarray(norm_w, dtype=np.float32)
    in_w = np.asarray(in_w, dtype=np.float32)
    in_b = np.asarray(in_b, dtype=np.float32)
    out_w = np.asarray(out_w, dtype=np.float32)
    out_b = np.asarray(out_b, dtype=np.float32)
    head_w = np.asarray(head_w, dtype=np.float32)
    head_b = np.asarray(head_b, dtype=np.float32)

    # per-vocab gate tables: everything upstream of the scan is token-pure
    var = (embed_w ** 2).mean(axis=1, keepdims=True)
    xn = embed_w / np.sqrt(var + EPS) * norm_w[None, :]     # [V, H]
    proj = xn @ in_w + in_b[None, :]                        # [V, 4S]
    xg = proj[:, 0 * S:1 * S]
    a_l = proj[:, 1 * S:2 * S]
    b_l = proj[:, 2 * S:3 * S]
    c_l = proj[:, 3 * S:4 * S]
    sig = lambda z: 1.0 / (1.0 + np.exp(-z))
    A = sig(a_l)                    # [V, S] forget gate
    BX = sig(b_l) * xg              # [V, S] input contribution
    C = sig(c_l)                    # [V, S] output gate
    LA = np.log(A)
    LC = np.log(C)
    CBX = C * BX                    # [V, S] gated input c*bx

    # two-hot gate-exponent operand: +1 at tok_t in the log(a) section and
    # the log(c) section, -1 at tok_{t-1} in the log(c) section (telescopes)
    ar = np.arange(BL)
    ohp = np.zeros((VP, BL), np.float32)
    ohp[tokens, ar] += 1.0                       # log(a) section
    ohp[V + tokens, ar] += 1.0                   # + log(c_t)
    nb = (ar % L) != 0                           # not a batch start
    ohp[V + tokens[ar[nb] - 1], ar[nb]] -= 1.0   # - log(c_{t-1})
    ohp = np.ascontiguousarray(ohp.astype(np.float16))

    CBXtok = CBX[tokens].astype(np.float16)      # [BL, S]
    outwh = out_w @ head_w                       # [S, V]

    in_maps = []
    for k in range(NCORES):
        ch0 = k * SS
        tab = np.zeros((VP, SS), np.float16)
        tab[:V] = LA[:, ch0:ch0 + SS].astype(np.float16)
        tab[V:2 * V] = LC[:, ch0:ch0 + SS].astype(np.float16)
        cc = CBXtok[:, ch0:ch0 + SS]             # [BL, SS]
        cbx_core = np.ascontiguousarray(
            cc.T.reshape(NST, 128, BL).transpose(1, 0, 2).reshape(128, NST * BL)
        )
        ow = outwh[ch0:ch0 + SS]                 # [SS, V]
        outwh_s = np.ascontiguousarray(
            ow.reshape(NST, 128, V).transpose(1, 0, 2).reshape(128, NST * V)
        ).astype(np.float16)
        in_maps.append({
            "ohp": ohp,
            "tab": tab,
            "cbx": cbx_core,
            "outwh": outwh_s,
        })

    # host epilogue: residual + biases commuted through the (linear) head
    emb_head = embed_w @ head_w                  # [V, V]
    res_logits = emb_head[tokens]                # [BL, V]
    bias_logits = out_b @ head_w + head_b        # [V]
    epilogue = (res_logits + bias_logits[None, :]).astype(np.float32)
    return in_maps, epilogue


def _finish(res, epilogue):
    total = np.zeros((V, BL), np.float32)
    for r in res.results:
        lg = np.asarray(r["logits"], dtype=np.float32)   # [128, BL//2]
        for b in range(NBLK):
            cols = slice(b * TC, (b + 1) * TC)
            total[:, (2 * b) * TC:(2 * b + 1) * TC] += lg[0:V, cols]
            total[:, (2 * b + 1) * TC:(2 * b + 2) * TC] += lg[64:64 + V, cols]
    out = total.T + epilogue
    return np.ascontiguousarray(out.reshape(B, L, V)).astype(np.float32)


def kernel(**inputs):
    in_maps, epilogue = _prep(**inputs)
    res = run_bass_kernel_spmd(_get_nc(), in_maps, core_ids=list(range(NCORES)))
    return _finish(res, epilogue)


def kernel_traced(**inputs):
    """Like kernel() but also returns the NTFF-profiled HW exec time (ns)."""
    in_maps, epilogue = _prep(**inputs)
    res = run_bass_kernel_spmd(
        _get_nc(), in_maps, core_ids=list(range(NCORES)), trace=True
    )
    return _finish(res, epilogue), res.exec_time_ns


# revision 21
# speedup vs baseline: 1.0098x; 1.0098x over previous
"""Trainium2 Bass kernel for nn_CopyModel (gated linear-recurrence LM block).

Model: embed -> rmsnorm -> in_proj(1024->4*4096) -> sigmoid gates ->
linear scan h_t = a_t*h_{t-1} + b_t*x_t -> out gate c_t*h_t ->
out_proj(4096->1024) + residual -> head(1024->62).

Key insights:
 1. The vocab has only 62 entries, so every per-token quantity (embed,
    rmsnorm, in_proj, gate sigmoids) is a table lookup. The host precomputes
    per-vocab tables; the device gathers rows via one-hot matmuls.
 2. The output gate folds INTO the scan in log domain: with
    z_t := c_t*h_t,  z_t = exp(la[tok_t] + lc[tok_t] - lc[tok_{t-1}]) * z_{t-1}
                           + (c*bx)[tok_t]
    so the device never multiplies by c at all. The gate exponent is ONE
    matmul per 128-channel tile: a 124-partition stationary stacks the
    log(a) and log(c) tables, and the moving "two-hot" holds +1 at tok_t
    (both sections) and -1 at tok_{t-1} (log(c) section). The log(c)
    quantization telescopes exactly (same fp16 entry +/-), and log(a)'s
    fp16 error vanishes as a->1, so precision is safe. exp() runs on the
    otherwise-idle Act engine.
 3. Everything downstream of z is linear: out_proj and head fuse into
    out_wh = out_w @ head_w [4096, 62]; residual + biases commute with the
    head into a host epilogue.

Sharding: STATE (4096) split 8 ways (512 channels/core), both batches on
every core; the host sums the 8 partial logit contributions.

Per core, per 512-token chunk (8 chunks): PE: 4 gate-gathers + 4 out
matmuls (all fp16, 1 cyc/row); Act: 4 exp()s + 1 logits copy; DVE: scans
in [128, 1024] blocks (the pacing engine, ~2 ALU-cycles/element); Pool:
idle; DMA: c*bx stream + logits out.
"""

import sys

for _p in ("/opt/trn_rl_repo",):
    if _p not in sys.path:
        sys.path.insert(0, _p)

import numpy as np

import concourse.bass as bass
import concourse.bacc as bacc
import concourse.tile as tile
from concourse import mybir
from concourse.bass_utils import run_bass_kernel_spmd

F32 = mybir.dt.float32
F16 = mybir.dt.float16
AF = mybir.ActivationFunctionType
OP = mybir.AluOpType

V = 62          # vocab
VP = 128        # vocab padded to full partition count
H = 1024        # hidden
S = 4096        # state
B, L = 2, 2048
BL = B * L      # 4096 tokens
NCORES = 8
SS = S // NCORES        # 512 state channels per core
NST = SS // 128         # 4 state tiles per core
TC = 512                # tokens per chunk
NCHUNK = BL // TC       # 8 chunks
NBLK = NCHUNK // 2      # 4 scan blocks of 1024 tokens (2 per batch)
EPS = 1e-6


def _build_nc():
    nc = bacc.Bacc("TRN2", target_bir_lowering=False, debug=False)

    ohp_d = nc.dram_tensor("ohp", [VP, BL], F16, kind="ExternalInput")
    tab_d = nc.dram_tensor("tab", [VP, SS], F16, kind="ExternalInput")
    cbx_d = nc.dram_tensor("cbx", [128, NST * BL], F16, kind="ExternalInput")
    outwh_d = nc.dram_tensor("outwh", [128, NST * V], F16, kind="ExternalInput")
    logits = nc.dram_tensor("logits", [128, BL // 2], F16, kind="ExternalOutput")

    with tile.TileContext(nc) as tc:
        with (
            tc.tile_pool(name="consts", bufs=1) as consts,
            tc.tile_pool(name="p_a", bufs=2) as p_a,
            tc.tile_pool(name="p_z", bufs=2) as p_z,
            tc.tile_pool(name="p_lg", bufs=2) as p_lg,
            tc.tile_pool(name="psG", bufs=3, space="PSUM") as psG,
            tc.tile_pool(name="psL", bufs=2, space="PSUM") as psL,
        ):
            # ---- loads, critical-path first ----
            tab = consts.tile([VP, SS], F16)
            ohp = consts.tile([VP, BL], F16)
            cbx = consts.tile([128, NST * BL], F16)
            outwh = consts.tile([128, NST * V], F16)
            nc.sync.dma_start(out=tab[:], in_=tab_d[:])
            nc.sync.dma_start(out=ohp[:, 0:2 * TC], in_=ohp_d[:, 0:2 * TC])
            for st in range(NST):
                nc.sync.dma_start(
                    out=cbx[:, st * BL:st * BL + 2 * TC],
                    in_=cbx_d[:, st * BL:st * BL + 2 * TC],
                )
            nc.sync.dma_start(out=outwh[:], in_=outwh_d[:])
            o = 2 * TC
            nc.sync.dma_start(out=ohp[:, o:BL], in_=ohp_d[:, o:BL])
            for st in range(NST):
                ob = st * BL + o
                nc.sync.dma_start(out=cbx[:, ob:ob + BL - o], in_=cbx_d[:, ob:ob + BL - o])

            # ---- PE warmup: burn the p-state ramp during the DMA preamble ----
            gw = consts.tile([128, TC], F16)
            nc.vector.memset(gw[:], 0.0)
            for i in range(2):
                wps = psG.tile([128, TC], F32, tag="g")
                nc.tensor.matmul(
                    wps[:, 0:TC // 2], gw[:, 0:128], gw[:, 0:TC // 2],
                    start=True, stop=True,
                )

            def emit_gather_exp(b, ap_tiles, split=False):
                for st in range(NST):
                    pg = psG.tile([128, 2 * TC], F32, tag="g", name=f"pg{st}")
                    for half in range(2):
                        t0 = (2 * b + half) * TC
                        hs = slice(half * TC, (half + 1) * TC)
                        nc.tensor.matmul(
                            pg[:, hs],
                            tab[:, st * 128:(st + 1) * 128], ohp[:, t0:t0 + TC],
                            start=True, stop=True,
                        )
                        if split:
                            nc.scalar.activation(ap_tiles[st][:, hs], pg[:, hs], AF.Exp)
                    if not split:
                        nc.scalar.activation(ap_tiles[st][:], pg[:], AF.Exp)

            def new_ap():
                return [p_a.tile([128, 2 * TC], F32, tag=f"ap{st}", name=f"ap{st}")
                        for st in range(NST)]

            def emit_outs(b, zt):
                # both chunks of the block into one psum bank: even chunk at
                # partitions 0..61, odd chunk at 64..125 (PE tile_position)
                pl = psL.tile([128, TC], F32, tag="l")
                for half in range(2):
                    pb = 64 * half
                    for st in range(NST):
                        nc.tensor.matmul(
                            pl[pb:pb + V, :], outwh[:, st * V:(st + 1) * V],
                            zt[st][:, half * TC:(half + 1) * TC],
                            start=(st == 0), stop=(st == NST - 1),
                        )
                lg = p_lg.tile([128, TC], F16, tag="lg")
                nc.gpsimd.memset(lg[:], 0.0)
                nc.scalar.activation(lg[0:V, :], pl[0:V, :], AF.Copy)
                nc.scalar.activation(lg[64:64 + V, :], pl[64:64 + V, :], AF.Copy)
                nc.sync.dma_start(out=logits[:, b * TC:(b + 1) * TC], in_=lg[:])

            ap_cur = new_ap()
            emit_gather_exp(0, ap_cur, split=True)
            prev_z = [None] * NST
            for b in range(NBLK):
                # scans for block b: z = gate*z_prev + cbx along 1024 tokens
                reset = (b % (NBLK // B)) == 0
                last = b == NBLK - 1
                zt = [p_z.tile([128, 2 * TC], F16, tag=f"z{st}", name=f"z{st}")
                      for st in range(NST)]
                halves = 2 if (b == 0 or last) else 1
                for half in range(halves):
                    hs = (slice(half * TC, (half + 1) * TC) if halves == 2
                          else slice(0, 2 * TC))
                    for st in range(NST):
                        if half == 0:
                            init = 0.0 if reset else prev_z[st][:, 2 * TC - 1:2 * TC]
                        else:
                            init = zt[st][:, TC - 1:TC]
                        nc.vector.tensor_tensor_scan(
                            zt[st][:, hs], ap_cur[st][:, hs],
                            cbx[:, st * BL + b * 2 * TC + hs.start:
                                 st * BL + b * 2 * TC + hs.stop],
                            init, op0=OP.mult, op1=OP.add,
                        )
                    if last and half == 0:
                        # tail shortening: chunk-6 outs run while the final
                        # scans execute
                        pl_t = psL.tile([128, TC], F32, tag="l")
                        for st in range(NST):
                            nc.tensor.matmul(
                                pl_t[0:V, :], outwh[:, st * V:(st + 1) * V],
                                zt[st][:, 0:TC],
                                start=(st == 0), stop=(st == NST - 1),
                            )
                        lg_t = p_lg.tile([128, TC], F16, tag="lg")
                        nc.gpsimd.memset(lg_t[:], 0.0)
                        nc.scalar.activation(lg_t[0:V, :], pl_t[0:V, :], AF.Copy)
                prev_z = zt
                # PE/Act run a block ahead while DVE scans
                ap_next = None
                if b + 1 < NBLK:
                    ap_next = new_ap()
                    emit_gather_exp(b + 1, ap_next)
                if last:
                    for st in range(NST):
                        nc.tensor.matmul(
                            pl_t[64:64 + V, :], outwh[:, st * V:(st + 1) * V],
                            zt[st][:, TC:2 * TC],
                            start=(st == 0), stop=(st == NST - 1),
                        )
                    nc.scalar.activation(lg_t[64:64 + V, :], pl_t[64:64 + V, :], AF.Copy)
                    nc.sync.dma_start(out=logits[:, b * TC:(b + 1) * TC], in_=lg_t[:])
                else:
                    emit_outs(b, zt)
                ap_cur = ap_next

    nc.compile()
    return nc


_NC = None


def _get_nc():
    global _NC
    if _NC is None:
        _NC = _build_nc()
    return _NC


def _prep(tokens, embed_w, norm_w, in_w, in_b, out_w, out_b, head_w, head_b):
    tokens = np.asarray(tokens).reshape(-1)
    embed_w = np.asarray(embed_w, dtype=np.float32)
    norm_w = np.asarray(norm_w, dtype=np.float32)
    in_w = np.asarray(in_w, dtype=np.float32)
    in_b = np.asarray(in_b, dtype=np.float32)
    out_w = np.asarray(out_w, dtype=np.float32)
    out_b = np.asarray(out_b, dtype=np.float32)
    head_w = np.asarray(head_w, dtype=np.float32)
    head_b = np.asarray(head_b, dtype=np.float32)

    # per-vocab gate tables: everything upstream of the scan is token-pure
    var = (embed_w ** 2).mean(axis=1, keepdims=True)
    xn = embed_w / np.sqrt(var + EPS) * norm_w[None, :]     # [V, H]
    proj = xn @ in_w + in_b[None, :]                        # [V, 4S]
    xg = proj[:, 0 * S:1 * S]
    a_l = proj[:, 1 * S:2 * S]
    b_l = proj[:, 2 * S:3 * S]
    c_l = proj[:, 3 * S:4 * S]
    sig = lambda z: 1.0 / (1.0 + np.exp(-z))
    A = sig(a_l)                    # [V, S] forget gate
    BX = sig(b_l) * xg              # [V, S] input contribution
    C = sig(c_l)                    # [V, S] output gate
    LA = np.log(A)
    LC = np.log(C)
    CBX = C * BX                    # [V, S] gated input c*bx

    # two-hot gate-exponent operand: +1 at tok_t in the log(a) section and
    # the log(c) section, -1 at tok_{t-1} in the log(c) section (telescopes)
    ar = np.arange(BL)
    ohp = np.zeros((VP, BL), np.float32)
    ohp[tokens, ar] += 1.0                       # log(a) section
    ohp[V + tokens, ar] += 1.0                   # + log(c_t)
    nb = (ar % L) != 0                           # not a batch start
    ohp[V + tokens[ar[nb] - 1], ar[nb]] -= 1.0   # - log(c_{t-1})
    ohp = np.ascontiguousarray(ohp.astype(np.float16))

    CBXtok = CBX[tokens].astype(np.float16)      # [BL, S]
    outwh = out_w @ head_w                       # [S, V]

    in_maps = []
    for k in range(NCORES):
        ch0 = k * SS
        tab = np.zeros((VP, SS), np.float16)
        tab[:V] = LA[:, ch0:ch0 + SS].astype(np.float16)
        tab[V:2 * V] = LC[:, ch0:ch0 + SS].astype(np.float16)
        cc = CBXtok[:, ch0:ch0 + SS]             # [BL, SS]
        cbx_core = np.ascontiguousarray(
            cc.T.reshape(NST, 128, BL).transpose(1, 0, 2).reshape(128, NST * BL)
        )
        ow = outwh[ch0:ch0 + SS]                 # [SS, V]
        outwh_s = np.ascontiguousarray(
            ow.reshape(NST, 128, V).transpose(1, 0, 2).reshape(128, NST * V)
        ).astype(np.float16)
        in_maps.append({
            "ohp": ohp,
            "tab": tab,
            "cbx": cbx_core,
            "outwh": outwh_s,
        })

    # host epilogue: residual + biases commuted through the (linear) head
    emb_head = embed_w @ head_w                  # [V, V]
    res_logits = emb_head[tokens]                # [BL, V]
    bias_logits = out_b @ head_w + head_b        # [V]
    epilogue = (res_logits + bias_logits[None, :]).astype(np.float32)
    return in_maps, epilogue


def _finish(res, epilogue):
    total = np.zeros((V, BL), np.float32)
    for r in res.results:
        lg = np.asarray(r["logits"], dtype=np.float32)   # [128, BL//2]
        for b in range(NBLK):
            cols = slice(b * TC, (b + 1) * TC)
            total[:, (2 * b) * TC:(2 * b + 1) * TC] += lg[0:V, cols]
            total[:, (2 * b + 1) * TC:(2 * b + 2) * TC] += lg[64:64 + V, cols]
    out = total.T + epilogue
    return np.ascontiguousarray(out.reshape(B, L, V)).astype(np.float32)


def kernel(**inputs):
    in_maps, epilogue = _prep(**inputs)
    res = run_bass_kernel_spmd(_get_nc(), in_maps, core_ids=list(range(NCORES)))
    return _finish(res, epilogue)


def kernel_traced(**inputs):
    """Like kernel() but also returns the NTFF-profiled HW exec time (ns)."""
    in_maps, epilogue = _prep(**inputs)
    res = run_bass_kernel_spmd(
        _get_nc(), in_maps, core_ids=list(range(NCORES)), trace=True
    )
    return _finish(res, epilogue), res.exec_time_ns
